# revision 26
# baseline (speedup 1.0000x reference)
"""Causal attention kernel for Trainium2 (Bass/Tile), 8-core data-parallel.

Problem: x [8, 2048, 1024] f32; W_query/W_key/W_value [1024, 1024] f32.
    q = x @ Wq; k = x @ Wk; v = x @ Wv       (per batch element)
    out = softmax(causal(q k^T) / 32) @ v
Sharding: batch dim (8) across the 8 NeuronCores, one element per core.

v2 design (vs the f32r/spill baseline):
  * All PE operands bf16 (PSUM accumulation stays f32): projections,
    scores, AV run at 1 cycle/row at ANY moving width; transposes at
    1 cycle/row via a bf16 identity. End-to-end rel err ~3.4e-3
    (harness gate 2e-2).
  * M-trick: scores = (x M) x^T with M = Wq Wk^T [1024x1024], computed
    once per core. Kills the k-projection (131k PE rows) for the cost
    of W transposes + a 1024^3 GEMM (98k rows), and makes x^T itself
    the score key operand.
  * Everything resident in SBUF as bf16 (xT 32K + q'T 32K + v 32K +
    M 16K per partition) -- no DRAM spill, no phase-boundary reload
    stall, no second pass over W.
  * Phase 1 ordered so DMA (20MB at ~330GB/s) stays ahead of the PE:
    Wk/Wq transposed first (f32 identity = slow fill while DMA ramps),
    then M, then x transposes interleaved with the q' projection
    windows, then the v projection.
"""

import os

import numpy as np

# Defensive: recover wedged cores at NRT/PJRT init (no-op on healthy devices).
os.environ.setdefault("NEURON_RT_RESET_CORES", "1")

import concourse.tile as tile
import concourse.mybir as mybir
from concourse import bacc, bass_utils
from concourse.masks import make_identity

F32 = mybir.dt.float32
F32R = mybir.dt.float32r
BF16 = mybir.dt.bfloat16
EXP = mybir.ActivationFunctionType.Exp
COPY = mybir.ActivationFunctionType.Copy
AXX = mybir.AxisListType.X

NTOK = 2048      # tokens per batch element (= per core)
D = 1024         # d_in = d_out
P = 128          # partitions
DC = D // P      # 8 d-chunks
NBLK = NTOK // P     # 16 token blocks
NJ = NTOK // 512     # 4 query chunks of 512
NEG = -1.0e9
SCALE = 1.0 / 32.0   # 1/sqrt(D)


def build_program():
    nc = bacc.Bacc("TRN2", target_bir_lowering=False, debug=False,
                   num_devices=8)
    x = nc.dram_tensor("x", [NTOK, D], F32, kind="ExternalInput").ap()
    wq = nc.dram_tensor("W_query", [D, D], F32, kind="ExternalInput").ap()
    wk = nc.dram_tensor("W_key", [D, D], F32, kind="ExternalInput").ap()
    wv = nc.dram_tensor("W_value", [D, D], F32, kind="ExternalInput").ap()
    out = nc.dram_tensor("out", [NTOK, D], F32, kind="ExternalOutput").ap()

    with tile.TileContext(nc) as tc:
        _emit(nc, tc, x, wq, wk, wv, out)
    nc.compile()
    return nc


class Feeder:
    """Issue DMA loads in a fixed order through a rotating tile pool,
    never more than `bufs` un-consumed loads in flight (so a buf is only
    recycled after its consumer instructions were emitted)."""

    def __init__(self, items, bufs):
        self.items = items          # list of (key, emit_fn)
        self.bufs = bufs
        self.issued = {}
        self.next_i = 0
        self.inflight = 0

    def pump(self, n=1):
        while (n > 0 and self.next_i < len(self.items)
               and self.inflight < self.bufs):
            key, fn = self.items[self.next_i]
            self.issued[key] = fn()
            self.next_i += 1
            self.inflight -= -1
            n -= 1

    def take(self, key):
        assert key in self.issued, f"load {key} not issued yet"
        self.inflight -= 1
        return self.issued.pop(key)


def _emit(nc, tc, x, wq, wk, wv, out):
    const = tc.alloc_tile_pool(name="const", bufs=1)
    resid = tc.alloc_tile_pool(name="resid", bufs=1)
    # PSUM: psA = 4 single-bank tiles (M/q' accum, scores, P^T transposes)
    #       psB = 2 double-bank tiles (ph1 transposes + v accum, ph2 AV accum)
    psA = tc.alloc_tile_pool(name="psA", bufs=2, space="PSUM")
    psB = tc.alloc_tile_pool(name="psB", bufs=3, space="PSUM")
    pden = tc.alloc_tile_pool(name="pden", bufs=3)

    # constants: identities (f32 for W fill-transposes, bf16 for the rest),
    # additive causal mask strip [zeros x384 | causal(128)]
    id32 = const.tile([P, P], F32, tag="id32")
    make_identity(nc, id32)
    idb = const.tile([P, P], BF16, tag="idb")
    nc.vector.tensor_copy(idb, id32)
    idr = const.tile([P, P], F32R, tag="idr")
    nc.vector.tensor_copy(idr, id32)
    maskA = const.tile([P, 512], F32, tag="maskA")
    nc.vector.memset(maskA[:, 0:384], 0.0)
    nc.vector.memset(maskA[:, 384:512], 0.0)
    nc.gpsimd.affine_select(
        out=maskA[:, 384:512], in_=maskA[:, 384:512],
        compare_op=mybir.AluOpType.is_ge, fill=NEG, base=0,
        pattern=[[-1, P]], channel_multiplier=1)

    def cb(sel, out_ap, in_ap):
        # split PSUM->SBUF copy traffic across ACT and DVE
        if sel % 2 == 0:
            nc.scalar.copy(out_ap, in_ap)
        else:
            nc.vector.tensor_copy(out_ap, in_ap)

    # residents (bf16): x^T, q'^T, v, M
    xT = resid.tile([P, DC, NTOK], BF16, tag="xT")    # [din%128, dc, tok]
    qT = resid.tile([P, DC, NTOK], BF16, tag="qT")    # [dout%128, dc, tok]
    vres = resid.tile([P, NBLK, D], BF16, tag="vres")  # [tok%128, blk, dout]
    Mt = resid.tile([P, DC, D], BF16, tag="Mt")       # M=WqWk^T [din%128, dc, din2]

    # ---------------- phase 1 ----------------
    ph1 = tc.alloc_tile_pool(name="ph1", bufs=1)
    wqrot = tc.alloc_tile_pool(name="wqrot", bufs=2)
    wdma = tc.alloc_tile_pool(name="wdma", bufs=3)
    xdma = tc.alloc_tile_pool(name="xdma", bufs=3)
    xb16 = tc.alloc_tile_pool(name="xb16", bufs=2)

    wkT = ph1.tile([P, DC, D], BF16, tag="wkT")   # Wk^T [dout%128, dc, din]
    wvb = ph1.tile([P, DC, D], BF16, tag="wvb")   # Wv   [din%128, dc, dout]

    def w_load(name, src, idx):
        def fn():
            t = wdma.tile([P, D], F32, tag="wdma", name=f"{name}{idx}")
            nc.sync.dma_start(out=t, in_=src[idx * P:(idx + 1) * P, :])
            return t
        return fn

    def x_load(idx):
        def fn():
            t = xdma.tile([P, D], F32, tag="xdma", name=f"x{idx}")
            nc.sync.dma_start(out=t, in_=x[idx * P:(idx + 1) * P, :])
            return t
        return fn

    wfeed = Feeder([(("wk", k), w_load("wk", wk, k)) for k in range(DC)]
                   + [(("wq", k), w_load("wq", wq, k)) for k in range(DC)]
                   + [(("wv", k), w_load("wv", wv, k)) for k in range(DC)],
                   bufs=3)
    xfeed = Feeder([(("x", t), x_load(t)) for t in range(NBLK)], bufs=3)

    wfeed.pump(3)

    # Wk transposes -- f32 identity on purpose: the PE is DMA-starved here,
    # so run the transposes in the slowest (2 cyc/row) mode to soak it up.
    for k in range(DC):
        t = wfeed.take(("wk", k))
        ps = psB.tile([P, D], F32, tag="psB", name=f"Twk{k}")
        for cc in range(DC):
            nc.tensor.transpose(
                ps[:, cc * P:(cc + 1) * P], t[:, cc * P:(cc + 1) * P], id32)
        cb(k, wkT[:, :, k * P:(k + 1) * P],
           ps.rearrange("p (c b) -> p c b", c=DC))
        wfeed.pump(1)

    # x transposes: cast to bf16 on the idle Pool engine first, then PE
    # transposes run at the bf16 rate (1 cyc/row) with 16-bit copies out
    def txpose(tb):
        t = xfeed.take(("x", tb))
        t16 = xb16.tile([P, D], BF16, tag="xb16", name=f"xb{tb}")
        nc.gpsimd.tensor_copy(t16, t)
        ps = psB.tile([P, D], BF16, tag="psB", name=f"Tx{tb}")
        for cc in range(DC):
            nc.tensor.transpose(
                ps[:, cc * P:(cc + 1) * P], t16[:, cc * P:(cc + 1) * P], idb)
        cb(tb, xT[:, :, tb * P:(tb + 1) * P],
           ps.rearrange("p (c t) -> p c t", c=DC))
        xfeed.pump(1)

    # Wq transposes + M = Wq Wk^T, software-pipelined one panel deep so
    # M(k) never waits on its wqp copy; x0..x3 transposed in the gaps
    xfeed.pump(3)

    def twq(k):
        t = wfeed.take(("wq", k))
        wqp = wqrot.tile([P, DC, P], BF16, tag="wqp")
        ps = psB.tile([P, D], F32, tag="psB", name=f"Twq{k}")
        for cc in range(DC):
            nc.tensor.transpose(
                ps[:, cc * P:(cc + 1) * P], t[:, cc * P:(cc + 1) * P], id32)
        cb(k, wqp, ps.rearrange("p (c b) -> p c b", c=DC))
        wfeed.pump(1)
        return wqp

    wqp_next = twq(0)
    for k in range(DC):
        wqp = wqp_next
        if k + 1 < DC:
            wqp_next = twq(k + 1)
        for ah in range(2):
            psm = psA.tile([P, 512], F32, tag="psA", name=f"M{k}h{ah}")
            for cc in range(DC):
                nc.tensor.matmul(
                    psm, wqp[:, cc, :], wkT[:, cc, ah * 512:(ah + 1) * 512],
                    start=(cc == 0), stop=(cc == DC - 1))
            cb(k + ah, Mt[:, k, ah * 512:(ah + 1) * 512], psm)
        if k >= 4:
            txpose(k - 4)

    # q' = x @ M, transposed output; each window's x transposes emitted a
    # window EARLY so their copies complete behind the previous window's
    # matmuls; two Wv casts (Pool engine) interleaved per window
    for tb in range(4, 8):
        txpose(tb)
    for w in range(NJ):
        for tb in range(4 * w + 8, 4 * w + 12):
            if tb < NBLK:
                txpose(tb)
        for k in (2 * w, 2 * w + 1):
            t = wfeed.take(("wv", k))
            nc.gpsimd.tensor_copy(wvb[:, k, :], t)
            wfeed.pump(1)
        for ab in range(DC):
            ps = psA.tile([P, 512], F32, tag="psA", name=f"q{w}a{ab}")
            for bc in range(DC):
                nc.tensor.matmul(
                    ps, Mt[:, bc, ab * P:(ab + 1) * P],
                    xT[:, bc, w * 512:(w + 1) * 512],
                    start=(bc == 0), stop=(bc == DC - 1))
            cb(w + ab, qT[:, ab, w * 512:(w + 1) * 512], ps)

    # v = x @ Wv in natural [tok, dout] layout (xT blocks stationary).
    # Copies go to the otherwise-idle Pool engine so ACT/DVE enter phase 2
    # with empty queues.
    for mb in range(NBLK):
        ps = psB.tile([P, D], F32, tag="psB", name=f"v{mb}")
        for h in range(2):
            for bc in range(DC):
                nc.tensor.matmul(
                    ps[:, h * 512:(h + 1) * 512], xT[:, bc, mb * P:(mb + 1) * P],
                    wvb[:, bc, h * 512:(h + 1) * 512],
                    start=(bc == 0), stop=(bc == DC - 1))
        cb(mb, vres[:, mb, :], ps)

    xb16.release()
    xdma.release()
    wdma.release()
    wqrot.release()
    ph1.release()

    # ---------------- phase 2: attention ----------------
    pexp = tc.alloc_tile_pool(name="pexp", bufs=4)
    ppt = tc.alloc_tile_pool(name="ppt", bufs=4)
    posb = tc.alloc_tile_pool(name="posb", bufs=2)

    # Work units (j, il, mc); software-pipelined one unit deep: the PE
    # transposes+AV of unit u-1 are emitted after the scores of unit u,
    # so the ACT exp of u-1 hides behind u's score matmuls.
    # group order: the single-unit j=0 groups are interleaved into the j=1
    # stream (their exp/copy latencies hide behind j=1's matmuls); the last
    # chunk runs big query blocks first so the kernel tail belongs to the
    # smallest diag unit
    groups = []
    for il in range(4):
        groups += [(1, il), (0, il)]
    groups += [(2, il) for il in range(4)]
    groups += [(3, il) for il in (3, 2, 1, 0)]
    units = []
    for j, il in groups:
        for mc in range(j + 1):
            units.append((j, il, mc))

    state = {}  # per-(j,il) live tiles: Ops, dpart

    def emit_scores(u):
        j, il, mc = u
        key = (j, il)
        if key not in state:
            state[key] = (psB.tile([P, D], F32, tag="psB", name="Ops"),
                          pden.tile([P, 6], F32, tag="dpart", name="dpart"))
        Ops, dpart = state[key]
        diag = (mc == j)
        wv_ = (il + 1) * P if diag else 512   # bf16: full rate at any width
        i = 4 * j + il
        sS = psA.tile([P, 512], F32, tag="psA", name="sS")
        for dc in range(DC):
            nc.tensor.matmul(
                sS[:, 0:wv_], qT[:, dc, i * P:(i + 1) * P],
                xT[:, dc, mc * 512:mc * 512 + wv_],
                start=(dc == 0), stop=(dc == DC - 1))
        if diag:
            s0 = (3 - il) * P
            nc.vector.tensor_add(sS[:, 0:wv_], sS[:, 0:wv_], maskA[:, s0:512])
        expP = pexp.tile([P, 512], BF16, tag="expP")
        nc.scalar.activation(expP[:, 0:wv_], sS[:, 0:wv_], EXP, scale=SCALE,
                             accum_out=dpart[:, mc:mc + 1])
        return expP

    def emit_pt(u, expP):
        # PE transposes of unit u's softmax weights + one strip copy; emitted
        # BEFORE the next unit's scores so the copy hides behind them
        j, il, mc = u
        nb = il + 1 if mc == j else 4
        ptp = psA.tile([P, 512], BF16, tag="psA")
        PT = ppt.tile([P, 512], BF16, tag="PT")
        for b in range(nb):
            nc.tensor.transpose(
                ptp[:, b * P:(b + 1) * P], expP[:, b * P:(b + 1) * P], idb)
        # strip copy split across both engines; block 0 lands first
        h = (nb + 1) // 2
        cb(4 * j + mc, PT[:, 0:h * P], ptp[:, 0:h * P])
        if nb > h:
            cb(4 * j + mc + 1, PT[:, h * P:nb * P], ptp[:, h * P:nb * P])
        return PT

    def emit_av(u, PT):
        j, il, mc = u
        Ops, dpart = state[(j, il)]
        nb = il + 1 if mc == j else 4
        if mc != j:
            for b in range(nb):
                mb = 4 * mc + b
                for hf in range(2):
                    nc.tensor.matmul(
                        Ops[:, hf * 512:(hf + 1) * 512],
                        PT[:, b * P:(b + 1) * P],
                        vres[:, mb, hf * 512:(hf + 1) * 512],
                        start=(mc == 0 and b == 0), stop=False)
        else:
            # final unit of query block i: den recip early, normalize halves
            # on both engines in parallel, store each half as it lands
            i = 4 * j + il
            den = pden.tile([P, 2], F32, tag="den")
            nc.vector.reduce_sum(out=den[:, 0:1], in_=dpart[:, 0:j + 1],
                                 axis=AXX)
            nc.vector.reciprocal(den[:, 1:2], den[:, 0:1])
            for b in range(nb):
                mb = 4 * mc + b
                for hf in range(2):
                    nc.tensor.matmul(
                        Ops[:, hf * 512:(hf + 1) * 512],
                        PT[:, b * P:(b + 1) * P],
                        vres[:, mb, hf * 512:(hf + 1) * 512],
                        start=(mc == 0 and b == 0), stop=(b == nb - 1))
            Osb = posb.tile([P, D], F32, tag="Osb")
            nc.vector.tensor_scalar_mul(Osb[:, 0:512], Ops[:, 0:512],
                                        den[:, 1:2])
            nc.scalar.activation(Osb[:, 512:1024], Ops[:, 512:1024], COPY,
                                 scale=den[:, 1:2])
            for hf in range(2):
                nc.sync.dma_start(
                    out=out[i * P:(i + 1) * P, hf * 512:(hf + 1) * 512],
                    in_=Osb[:, hf * 512:(hf + 1) * 512])
            del state[(j, il)]

    prev = None  # (unit, expP)
    for u in units:
        if prev is not None:
            pt = emit_pt(*prev)
        expP = emit_scores(u)
        if prev is not None:
            emit_av(prev[0], pt)
        prev = (u, expP)
    pt = emit_pt(*prev)
    emit_av(prev[0], pt)

    for pool in (posb, ppt, pexp, pden, psB, psA, resid, const):
        pool.release()


_NC_CACHE = None


def _get_nc():
    global _NC_CACHE
    if _NC_CACHE is None:
        _NC_CACHE = build_program()
    return _NC_CACHE


def kernel(x, W_query, W_key, W_value):
    """Full causal attention: x [8, 2048, 1024] -> [8, 2048, 1024] (f32)."""
    nc = _get_nc()
    x = np.ascontiguousarray(np.asarray(x, dtype=np.float32))
    wq = np.ascontiguousarray(np.asarray(W_query, dtype=np.float32))
    wk = np.ascontiguousarray(np.asarray(W_key, dtype=np.float32))
    wv = np.ascontiguousarray(np.asarray(W_value, dtype=np.float32))
    n_cores = x.shape[0]
    in_maps = [
        {"x": x[b], "W_query": wq, "W_key": wk, "W_value": wv}
        for b in range(n_cores)
    ]
    res = bass_utils.run_bass_kernel_spmd(nc, in_maps,
                                          core_ids=list(range(n_cores)))
    return np.stack([res.results[b]["out"] for b in range(n_cores)])


# revision 30
# speedup vs baseline: 1.0106x; 1.0106x over previous
"""Causal attention kernel for Trainium2 (Bass/Tile), 8-core data-parallel.

Problem: x [8, 2048, 1024] f32; W_query/W_key/W_value [1024, 1024] f32.
    q = x @ Wq; k = x @ Wk; v = x @ Wv       (per batch element)
    out = softmax(causal(q k^T) / 32) @ v
Sharding: batch dim (8) across the 8 NeuronCores, one element per core.

v2 design (vs the f32r/spill baseline):
  * All PE operands bf16 (PSUM accumulation stays f32): projections,
    scores, AV run at 1 cycle/row at ANY moving width; transposes at
    1 cycle/row via a bf16 identity. End-to-end rel err ~3.4e-3
    (harness gate 2e-2).
  * M-trick: scores = (x M) x^T with M = Wq Wk^T [1024x1024], computed
    once per core. Kills the k-projection (131k PE rows) for the cost
    of W transposes + a 1024^3 GEMM (98k rows), and makes x^T itself
    the score key operand.
  * Everything resident in SBUF as bf16 (xT 32K + q'T 32K + v 32K +
    M 16K per partition) -- no DRAM spill, no phase-boundary reload
    stall, no second pass over W.
  * Phase 1 ordered so DMA (20MB at ~330GB/s) stays ahead of the PE:
    Wk/Wq transposed first (f32 identity = slow fill while DMA ramps),
    then M, then x transposes interleaved with the q' projection
    windows, then the v projection.
"""

import os

import numpy as np

# Defensive: recover wedged cores at NRT/PJRT init (no-op on healthy devices).
os.environ.setdefault("NEURON_RT_RESET_CORES", "1")

import concourse.tile as tile
import concourse.mybir as mybir
from concourse import bacc, bass_utils
from concourse.masks import make_identity

F32 = mybir.dt.float32
F32R = mybir.dt.float32r
BF16 = mybir.dt.bfloat16
EXP = mybir.ActivationFunctionType.Exp
COPY = mybir.ActivationFunctionType.Copy
AXX = mybir.AxisListType.X

NTOK = 2048      # tokens per batch element (= per core)
D = 1024         # d_in = d_out
P = 128          # partitions
DC = D // P      # 8 d-chunks
NBLK = NTOK // P     # 16 token blocks
NJ = NTOK // 512     # 4 query chunks of 512
NEG = -1.0e9
SCALE = 1.0 / 32.0   # 1/sqrt(D)


def build_program():
    nc = bacc.Bacc("TRN2", target_bir_lowering=False, debug=False,
                   num_devices=8)
    x = nc.dram_tensor("x", [NTOK, D], F32, kind="ExternalInput").ap()
    wq = nc.dram_tensor("W_query", [D, D], F32, kind="ExternalInput").ap()
    wk = nc.dram_tensor("W_key", [D, D], F32, kind="ExternalInput").ap()
    wv = nc.dram_tensor("W_value", [D, D], F32, kind="ExternalInput").ap()
    out = nc.dram_tensor("out", [NTOK, D], F32, kind="ExternalOutput").ap()

    with tile.TileContext(nc) as tc:
        _emit(nc, tc, x, wq, wk, wv, out)
    nc.compile()
    return nc


class Feeder:
    """Issue DMA loads in a fixed order through a rotating tile pool,
    never more than `bufs` un-consumed loads in flight (so a buf is only
    recycled after its consumer instructions were emitted)."""

    def __init__(self, items, bufs):
        self.items = items          # list of (key, emit_fn)
        self.bufs = bufs
        self.issued = {}
        self.next_i = 0
        self.inflight = 0

    def pump(self, n=1):
        while (n > 0 and self.next_i < len(self.items)
               and self.inflight < self.bufs):
            key, fn = self.items[self.next_i]
            self.issued[key] = fn()
            self.next_i += 1
            self.inflight -= -1
            n -= 1

    def take(self, key):
        assert key in self.issued, f"load {key} not issued yet"
        self.inflight -= 1
        return self.issued.pop(key)


def _emit(nc, tc, x, wq, wk, wv, out):
    const = tc.alloc_tile_pool(name="const", bufs=1)
    resid = tc.alloc_tile_pool(name="resid", bufs=1)
    # PSUM: psA = 4 single-bank tiles (M/q' accum, scores, P^T transposes)
    #       psB = 2 double-bank tiles (ph1 transposes + v accum, ph2 AV accum)
    psA = tc.alloc_tile_pool(name="psA", bufs=2, space="PSUM")
    psB = tc.alloc_tile_pool(name="psB", bufs=3, space="PSUM")
    pden = tc.alloc_tile_pool(name="pden", bufs=3)

    # constants: identities (f32 for W fill-transposes, bf16 for the rest),
    # additive causal mask strip [zeros x384 | causal(128)]
    id32 = const.tile([P, P], F32, tag="id32")
    make_identity(nc, id32)
    idb = const.tile([P, P], BF16, tag="idb")
    nc.vector.tensor_copy(idb, id32)
    idr = const.tile([P, P], F32R, tag="idr")
    nc.vector.tensor_copy(idr, id32)
    maskA = const.tile([P, 512], F32, tag="maskA")
    nc.vector.memset(maskA[:, 0:384], 0.0)
    nc.vector.memset(maskA[:, 384:512], 0.0)
    nc.gpsimd.affine_select(
        out=maskA[:, 384:512], in_=maskA[:, 384:512],
        compare_op=mybir.AluOpType.is_ge, fill=NEG, base=0,
        pattern=[[-1, P]], channel_multiplier=1)

    def cb(sel, out_ap, in_ap):
        # split PSUM->SBUF copy traffic across ACT and DVE
        if sel % 2 == 0:
            nc.scalar.copy(out_ap, in_ap)
        else:
            nc.vector.tensor_copy(out_ap, in_ap)

    # residents (bf16): x^T, q'^T, v, M
    xT = resid.tile([P, DC, NTOK], BF16, tag="xT")    # [din%128, dc, tok]
    qT = resid.tile([P, DC, NTOK], BF16, tag="qT")    # [dout%128, dc, tok]
    vres = resid.tile([P, NBLK, D], BF16, tag="vres")  # [tok%128, blk, dout]
    Mt = resid.tile([P, DC, D], BF16, tag="Mt")       # M=WqWk^T [din%128, dc, din2]

    # ---------------- phase 1 ----------------
    ph1 = tc.alloc_tile_pool(name="ph1", bufs=1)
    wqrot = tc.alloc_tile_pool(name="wqrot", bufs=2)
    wdma = tc.alloc_tile_pool(name="wdma", bufs=3)
    xdma = tc.alloc_tile_pool(name="xdma", bufs=3)

    wkT = ph1.tile([P, DC, D], BF16, tag="wkT")   # Wk^T [dout%128, dc, din]
    wvb = ph1.tile([P, DC, D], BF16, tag="wvb")   # Wv   [din%128, dc, dout]

    def w_load(name, src, idx):
        def fn():
            t = wdma.tile([P, D], F32, tag="wdma", name=f"{name}{idx}")
            nc.sync.dma_start(out=t, in_=src[idx * P:(idx + 1) * P, :])
            return t
        return fn

    def x_load(idx):
        def fn():
            t = xdma.tile([P, D], F32R, tag="xdma", name=f"x{idx}")
            nc.sync.dma_start(
                out=t, in_=x[idx * P:(idx + 1) * P, :].bitcast(F32R))
            return t
        return fn

    wfeed = Feeder([(("wk", k), w_load("wk", wk, k)) for k in range(DC)]
                   + [(("wq", k), w_load("wq", wq, k)) for k in range(DC)]
                   + [(("wv", k), w_load("wv", wv, k)) for k in range(DC)],
                   bufs=3)
    xfeed = Feeder([(("x", t), x_load(t)) for t in range(NBLK)], bufs=3)

    wfeed.pump(3)

    # Wk transposes -- f32 identity on purpose: the PE is DMA-starved here,
    # so run the transposes in the slowest (2 cyc/row) mode to soak it up.
    for k in range(DC):
        t = wfeed.take(("wk", k))
        ps = psB.tile([P, D], F32, tag="psB", name=f"Twk{k}")
        for cc in range(DC):
            nc.tensor.transpose(
                ps[:, cc * P:(cc + 1) * P], t[:, cc * P:(cc + 1) * P], id32)
        cb(k, wkT[:, :, k * P:(k + 1) * P],
           ps.rearrange("p (c b) -> p c b", c=DC))
        wfeed.pump(1)

    # x transposes: f32r identity (1.5 cyc/row; the hw compiler rejects a
    # bf16 identity against f32r data); output copy casts to bf16
    def txpose(tb):
        t = xfeed.take(("x", tb))
        ps = psB.tile([P, D], F32, tag="psB", name=f"Tx{tb}")
        psr = ps.bitcast(F32R)
        for cc in range(DC):
            nc.tensor.transpose(
                psr[:, cc * P:(cc + 1) * P], t[:, cc * P:(cc + 1) * P], idr)
        cb(tb, xT[:, :, tb * P:(tb + 1) * P],
           ps.rearrange("p (c t) -> p c t", c=DC))
        xfeed.pump(1)

    # Wq transposes + M = Wq Wk^T, software-pipelined one panel deep so
    # M(k) never waits on its wqp copy; x0..x3 transposed in the gaps
    xfeed.pump(3)

    def twq(k):
        t = wfeed.take(("wq", k))
        wqp = wqrot.tile([P, DC, P], BF16, tag="wqp")
        ps = psB.tile([P, D], F32, tag="psB", name=f"Twq{k}")
        for cc in range(DC):
            nc.tensor.transpose(
                ps[:, cc * P:(cc + 1) * P], t[:, cc * P:(cc + 1) * P], id32)
        cb(k, wqp, ps.rearrange("p (c b) -> p c b", c=DC))
        wfeed.pump(1)
        return wqp

    wqp_next = twq(0)
    for k in range(DC):
        wqp = wqp_next
        if k + 1 < DC:
            wqp_next = twq(k + 1)
        for ah in range(2):
            psm = psA.tile([P, 512], F32, tag="psA", name=f"M{k}h{ah}")
            for cc in range(DC):
                nc.tensor.matmul(
                    psm, wqp[:, cc, :], wkT[:, cc, ah * 512:(ah + 1) * 512],
                    start=(cc == 0), stop=(cc == DC - 1))
            cb(k + ah, Mt[:, k, ah * 512:(ah + 1) * 512], psm)
        if k >= 4:
            txpose(k - 4)

    # q' = x @ M, transposed output; each window's x transposes emitted a
    # window EARLY so their copies complete behind the previous window's
    # matmuls; two Wv casts (Pool engine) interleaved per window
    for tb in range(4, 8):
        txpose(tb)
    for w in range(NJ):
        for tb in range(4 * w + 8, 4 * w + 12):
            if tb < NBLK:
                txpose(tb)
        for k in (2 * w, 2 * w + 1):
            t = wfeed.take(("wv", k))
            nc.gpsimd.tensor_copy(wvb[:, k, :], t)
            wfeed.pump(1)
        for ab in range(DC):
            ps = psA.tile([P, 512], F32, tag="psA", name=f"q{w}a{ab}")
            for bc in range(DC):
                nc.tensor.matmul(
                    ps, Mt[:, bc, ab * P:(ab + 1) * P],
                    xT[:, bc, w * 512:(w + 1) * 512],
                    start=(bc == 0), stop=(bc == DC - 1))
            cb(w + ab, qT[:, ab, w * 512:(w + 1) * 512], ps)

    # v = x @ Wv in natural [tok, dout] layout (xT blocks stationary).
    # Copies go to the otherwise-idle Pool engine so ACT/DVE enter phase 2
    # with empty queues.
    for mb in range(NBLK):
        ps = psB.tile([P, D], F32, tag="psB", name=f"v{mb}")
        for h in range(2):
            for bc in range(DC):
                nc.tensor.matmul(
                    ps[:, h * 512:(h + 1) * 512], xT[:, bc, mb * P:(mb + 1) * P],
                    wvb[:, bc, h * 512:(h + 1) * 512],
                    start=(bc == 0), stop=(bc == DC - 1))
        cb(mb, vres[:, mb, :], ps)

    xdma.release()
    wdma.release()
    wqrot.release()
    ph1.release()

    # ---------------- phase 2: attention ----------------
    pexp = tc.alloc_tile_pool(name="pexp", bufs=4)
    ppt = tc.alloc_tile_pool(name="ppt", bufs=4)
    posb = tc.alloc_tile_pool(name="posb", bufs=2)

    # Work units (j, il, mc); software-pipelined one unit deep: the PE
    # transposes+AV of unit u-1 are emitted after the scores of unit u,
    # so the ACT exp of u-1 hides behind u's score matmuls.
    # group order: the single-unit j=0 groups are interleaved into the j=1
    # stream (their exp/copy latencies hide behind j=1's matmuls); the last
    # chunk runs big query blocks first so the kernel tail belongs to the
    # smallest diag unit
    groups = []
    for il in range(4):
        groups += [(1, il), (0, il)]
    groups += [(2, il) for il in range(4)]
    groups += [(3, il) for il in (3, 2, 1, 0)]
    units = []
    for j, il in groups:
        for mc in range(j + 1):
            units.append((j, il, mc))

    state = {}  # per-(j,il) live tiles: Ops, dpart

    def emit_scores(u):
        j, il, mc = u
        key = (j, il)
        if key not in state:
            state[key] = (psB.tile([P, D], F32, tag="psB", name="Ops"),
                          pden.tile([P, 6], F32, tag="dpart", name="dpart"))
        Ops, dpart = state[key]
        diag = (mc == j)
        wv_ = (il + 1) * P if diag else 512   # bf16: full rate at any width
        i = 4 * j + il
        sS = psA.tile([P, 512], F32, tag="psA", name="sS")
        for dc in range(DC):
            nc.tensor.matmul(
                sS[:, 0:wv_], qT[:, dc, i * P:(i + 1) * P],
                xT[:, dc, mc * 512:mc * 512 + wv_],
                start=(dc == 0), stop=(dc == DC - 1))
        if diag:
            s0 = (3 - il) * P
            nc.vector.tensor_add(sS[:, 0:wv_], sS[:, 0:wv_], maskA[:, s0:512])
        expP = pexp.tile([P, 512], BF16, tag="expP")
        nc.scalar.activation(expP[:, 0:wv_], sS[:, 0:wv_], EXP, scale=SCALE,
                             accum_out=dpart[:, mc:mc + 1])
        return expP

    def emit_pt(u, expP):
        # PE transposes of unit u's softmax weights + one strip copy; emitted
        # BEFORE the next unit's scores so the copy hides behind them
        j, il, mc = u
        nb = il + 1 if mc == j else 4
        ptp = psA.tile([P, 512], BF16, tag="psA")
        PT = ppt.tile([P, 512], BF16, tag="PT")
        for b in range(nb):
            nc.tensor.transpose(
                ptp[:, b * P:(b + 1) * P], expP[:, b * P:(b + 1) * P], idb)
        # strip copy split across both engines; block 0 lands first
        h = (nb + 1) // 2
        cb(4 * j + mc, PT[:, 0:h * P], ptp[:, 0:h * P])
        if nb > h:
            cb(4 * j + mc + 1, PT[:, h * P:nb * P], ptp[:, h * P:nb * P])
        return PT

    def emit_av(u, PT):
        j, il, mc = u
        Ops, dpart = state[(j, il)]
        nb = il + 1 if mc == j else 4
        if mc != j:
            for b in range(nb):
                mb = 4 * mc + b
                for hf in range(2):
                    nc.tensor.matmul(
                        Ops[:, hf * 512:(hf + 1) * 512],
                        PT[:, b * P:(b + 1) * P],
                        vres[:, mb, hf * 512:(hf + 1) * 512],
                        start=(mc == 0 and b == 0), stop=False)
        else:
            # final unit of query block i: den recip early, normalize halves
            # on both engines in parallel, store each half as it lands
            i = 4 * j + il
            den = pden.tile([P, 2], F32, tag="den")
            nc.vector.reduce_sum(out=den[:, 0:1], in_=dpart[:, 0:j + 1],
                                 axis=AXX)
            nc.vector.reciprocal(den[:, 1:2], den[:, 0:1])
            for b in range(nb):
                mb = 4 * mc + b
                for hf in range(2):
                    nc.tensor.matmul(
                        Ops[:, hf * 512:(hf + 1) * 512],
                        PT[:, b * P:(b + 1) * P],
                        vres[:, mb, hf * 512:(hf + 1) * 512],
                        start=(mc == 0 and b == 0), stop=(b == nb - 1))
            Osb = posb.tile([P, D], F32, tag="Osb")
            nc.vector.tensor_scalar_mul(Osb[:, 0:512], Ops[:, 0:512],
                                        den[:, 1:2])
            nc.scalar.activation(Osb[:, 512:1024], Ops[:, 512:1024], COPY,
                                 scale=den[:, 1:2])
            for hf in range(2):
                nc.sync.dma_start(
                    out=out[i * P:(i + 1) * P, hf * 512:(hf + 1) * 512],
                    in_=Osb[:, hf * 512:(hf + 1) * 512])
            del state[(j, il)]

    prev = None  # (unit, expP)
    for u in units:
        if prev is not None:
            pt = emit_pt(*prev)
        expP = emit_scores(u)
        if prev is not None:
            emit_av(prev[0], pt)
        prev = (u, expP)
    pt = emit_pt(*prev)
    emit_av(prev[0], pt)

    for pool in (posb, ppt, pexp, pden, psB, psA, resid, const):
        pool.release()


_NC_CACHE = None


def _get_nc():
    global _NC_CACHE
    if _NC_CACHE is None:
        _NC_CACHE = build_program()
    return _NC_CACHE


def kernel(x, W_query, W_key, W_value):
    """Full causal attention: x [8, 2048, 1024] -> [8, 2048, 1024] (f32)."""
    nc = _get_nc()
    x = np.ascontiguousarray(np.asarray(x, dtype=np.float32))
    wq = np.ascontiguousarray(np.asarray(W_query, dtype=np.float32))
    wk = np.ascontiguousarray(np.asarray(W_key, dtype=np.float32))
    wv = np.ascontiguousarray(np.asarray(W_value, dtype=np.float32))
    n_cores = x.shape[0]
    in_maps = [
        {"x": x[b], "W_query": wq, "W_key": wk, "W_value": wv}
        for b in range(n_cores)
    ]
    res = bass_utils.run_bass_kernel_spmd(nc, in_maps,
                                          core_ids=list(range(n_cores)))
    return np.stack([res.results[b]["out"] for b in range(n_cores)])
